# revision 8
# baseline (speedup 1.0000x reference)
"""GCNConv Trainium2 kernel: out = D^{-1/2} A D^{-1/2} (X @ W).

Strategy (8 NeuronCores, 1D row partition of the uniform-degree CSR):
  - each core owns 12500 destination nodes (padded to 12544 = 98*128)
  - phase A: X_k @ W, row-scaled by d_j  ->  X'' shard  (PE matmul,
    PE transpose for the K-major layout)
  - phase B: AllGather X'' shards -> full [100352, 64] table per core
  - phase C: per 128-node tile, one indirect DMA gathers the 16 neighbor
    rows of each node (2048 rows x 256B), DVE reduces the 16 segments,
    scales by d_i, stores.
Host side: shard/pad inputs, remap column indices into the padded
AllGather layout, unshard the output.
"""

import numpy as np

N_NODES = 100000
D_IN = 256
D_OUT = 64
DEG = 16
N_CORES = 8
P = 128
SHARD = N_NODES // N_CORES            # 12500
N_TILES = (SHARD + P - 1) // P        # 98
NPAD = N_TILES * P                    # 12544

_CACHE = {}


def _build_program(n_tiles=N_TILES, deg=DEG, d_in=D_IN, d_out=D_OUT,
                   n_cores=N_CORES, debug_taps=False):
    import concourse.bacc as bacc
    from concourse import bass, mybir, tile
    from concourse.masks import make_identity

    npad = n_tiles * P
    f32 = mybir.dt.float32

    nc = bacc.Bacc("TRN2", target_bir_lowering=False, debug=False,
                   num_devices=n_cores)
    Xs = nc.dram_tensor("Xs", [npad, d_in], f32, kind="ExternalInput").ap()
    W = nc.dram_tensor("W", [d_in, d_out], f32, kind="ExternalInput").ap()
    degs = nc.dram_tensor("degs", [npad, 1], f32, kind="ExternalInput").ap()
    idxs = nc.dram_tensor("idxs", [deg, npad], mybir.dt.int32,
                          kind="ExternalInput").ap()
    out = nc.dram_tensor("out", [npad, d_out], f32, kind="ExternalOutput").ap()
    if debug_taps:
        xpd_out = nc.dram_tensor("xpd_out", [npad, d_out], f32,
                                 kind="ExternalOutput").ap()
        xfull_out = nc.dram_tensor("xfull_out", [n_cores * npad, d_out], f32,
                                   kind="ExternalOutput").ap()

    n_kchunk = d_in // P  # 2

    with tile.TileContext(nc) as tc:
        with (
            tc.tile_pool(name="const", bufs=1) as constp,
            tc.tile_pool(name="xin", bufs=3) as xinp,
            tc.tile_pool(name="xtr", bufs=3) as xtp,
            tc.tile_pool(name="ps", bufs=2, space="PSUM") as psp,
            tc.tile_pool(name="xp", bufs=3) as xpp,
            tc.tile_pool(name="dg", bufs=3) as degp,
            tc.tile_pool(name="ix", bufs=8) as idxp,
            tc.tile_pool(name="gt", bufs=4) as gp,
            tc.tile_pool(name="ot", bufs=3) as outp,
            tc.tile_pool(name="dram", bufs=1, space="DRAM") as dramp,
        ):
            identity = constp.tile([P, P], f32)
            make_identity(nc, identity[:])
            w_sb = constp.tile([P, n_kchunk * d_out], f32)
            for c in range(n_kchunk):
                nc.sync.dma_start(out=w_sb[:, c * d_out:(c + 1) * d_out],
                                  in_=W[c * P:(c + 1) * P, :])

            xpd = dramp.tile([npad, d_out], f32)
            xfull = dramp.tile([n_cores * npad, d_out], f32,
                               addr_space="Shared")

            # ---- Phase A: X'' = (X @ W) * d ----
            for t in range(n_tiles):
                sl = slice(t * P, (t + 1) * P)
                xt_t = xinp.tile([P, d_in], f32)
                nc.sync.dma_start(out=xt_t[:], in_=Xs[sl, :])
                deg_t = degp.tile([P, 1], f32)
                nc.sync.dma_start(out=deg_t[:], in_=degs[sl, :])
                pso = psp.tile([P, d_out], f32, space="PSUM")
                for c in range(n_kchunk):
                    psT = psp.tile([P, P], f32, space="PSUM", tag="psT")
                    nc.tensor.transpose(psT[:], xt_t[:, c * P:(c + 1) * P],
                                        identity[:])
                    xT = xtp.tile([P, P], f32, tag="xT")
                    if c % 2 == 0:
                        nc.scalar.copy(xT[:], psT[:])
                    else:
                        nc.vector.tensor_copy(xT[:], psT[:])
                    nc.tensor.matmul(pso[:], xT[:],
                                     w_sb[:, c * d_out:(c + 1) * d_out],
                                     start=(c == 0), stop=(c == n_kchunk - 1))
                xp_t = xpp.tile([P, d_out], f32)
                nc.vector.tensor_scalar_mul(xp_t[:], pso[:], deg_t[:, 0:1])
                nc.sync.dma_start(out=xpd[sl, :], in_=xp_t[:])

            # ---- Phase B: AllGather shards ----
            nc.gpsimd.collective_compute(
                "AllGather", mybir.AluOpType.bypass,
                replica_groups=[list(range(n_cores))],
                ins=[xpd.opt()], outs=[xfull.opt()],
            )

            if debug_taps:
                nc.sync.dma_start(out=xpd_out[:], in_=xpd[:])
                nc.sync.dma_start(out=xfull_out[:], in_=xfull[:])

            # ---- Phase C: gather neighbors (one indirect DMA per slot,
            # HW supports one index per partition) + reduce ----
            for t in range(n_tiles):
                sl = slice(t * P, (t + 1) * P)
                g_t = gp.tile([P, deg * d_out], f32)
                for s in range(deg):
                    idx_t = idxp.tile([P, 1], mybir.dt.int32, tag="idx")
                    nc.sync.dma_start(out=idx_t[:], in_=idxs[s, sl][:, None])
                    nc.gpsimd.indirect_dma_start(
                        out=g_t[:, s * d_out:(s + 1) * d_out], out_offset=None,
                        in_=xfull[:],
                        in_offset=bass.IndirectOffsetOnAxis(ap=idx_t[:],
                                                            axis=0),
                    )
                deg_c = degp.tile([P, 1], f32, tag="deg_c")
                nc.sync.dma_start(out=deg_c[:], in_=degs[sl, :])
                r_t = outp.tile([P, d_out], f32, tag="r_t")
                nc.vector.tensor_reduce(
                    r_t[:], g_t[:].rearrange("p (s f) -> p f s", s=deg),
                    axis=mybir.AxisListType.X, op=mybir.AluOpType.add)
                o_t = outp.tile([P, d_out], f32, tag="o_t")
                nc.vector.tensor_scalar_mul(o_t[:], r_t[:], deg_c[:, 0:1])
                nc.sync.dma_start(out=out[sl, :], in_=o_t[:])

    nc.compile()
    return nc


def _get_program():
    key = "main"
    if key not in _CACHE:
        _CACHE[key] = _build_program()
    return _CACHE[key]


def _prep_inputs(X, weights, column_index, degrees,
                 n_nodes=N_NODES, n_cores=N_CORES, shard=SHARD, npad=NPAD,
                 deg=DEG):
    """Shard + pad host arrays; remap columns to padded AllGather layout."""
    X = np.ascontiguousarray(np.asarray(X, dtype=np.float32))
    W = np.ascontiguousarray(np.asarray(weights, dtype=np.float32))
    col = np.asarray(column_index).astype(np.int64, copy=False)
    dg = np.asarray(degrees, dtype=np.float32)

    # remap node id -> row in the AllGather-concatenated padded table
    col32 = (col // shard * npad + col % shard).astype(np.int32)
    col32 = col32.reshape(n_cores, shard, deg)

    in_maps = []
    pad = npad - shard
    for c in range(n_cores):
        Xc = np.concatenate(
            [X[c * shard:(c + 1) * shard],
             np.zeros((pad, X.shape[1]), np.float32)], axis=0)
        dgc = np.concatenate(
            [dg[c * shard:(c + 1) * shard],
             np.zeros(pad, np.float32)], axis=0).reshape(npad, 1)
        ixc = np.concatenate(
            [col32[c], np.zeros((pad, deg), np.int32)], axis=0)
        in_maps.append({"Xs": Xc, "W": W, "degs": dgc,
                        "idxs": np.ascontiguousarray(ixc.T)})
    return in_maps


def kernel(X, weights, row_pointers, column_index, degrees):
    from concourse.bass_utils import run_bass_kernel_spmd

    rp = np.asarray(row_pointers)
    assert rp.shape[0] == N_NODES + 1
    in_maps = _prep_inputs(X, weights, column_index, degrees)
    nc = _get_program()
    res = run_bass_kernel_spmd(nc, in_maps, core_ids=list(range(N_CORES)))
    outs = [res.results[c]["out"][:SHARD] for c in range(N_CORES)]
    return np.concatenate(outs, axis=0)
